# revision 11
# baseline (speedup 1.0000x reference)
"""Trainium2 Bass kernel for DecoderWithAttention (bidirectional 2-layer LSTM +
additive attention + gated fc), data-parallel over batch across 8 NeuronCores.

Shapes (hardcoded): encoder_out (64, 512, 16, 16), T=16, D=A=512, V=5000.
Per core: 8 batches, full network, weights replicated (no collectives available
under this axon terminal, so each core is fully independent).

Key layout decisions (per core):
  - All matmuls weight-stationary: matmul(out, lhsT, rhs): out = lhsT^T @ rhs.
  - LSTM gates PSUM: [128 part = gate%128, cols = (gate_chunk 16, batch 8)].
  - Input projections for all 16 steps batched (N=128); only Whh per step.
  - Hidden stores H*: [128, dch(4), t(16), b(8)] bf16, logical-t order (the
    reverse cells index t=15-s at compile time, so no data reversal anywhere).
  - Attention in transposed layout (A on partitions). relu(x)@Wfull uses
    relu(x)*w = sgn(w)*relu(x*|w|): |w| folded into ACT scale / precomputed
    tiles, sgn(w) as the PE reduction rhs. Softmax over p via PE ones-sum in
    [p, (b,t)] layout, no max subtraction (|score| bounded), bfull dropped
    (softmax shift invariance).
  - gate softmax(2) == sigmoid(logit diff), Wg[0]-Wg[1] folded host-side.
  - Mean over H folded into Wih1 (1/16); bih+bhh folded host-side.
"""

import numpy as np
import ml_dtypes

BF = ml_dtypes.bfloat16
B, E, HH, WW = 64, 512, 16, 16
T = WW          # 16 timesteps
PP = HH * WW    # 256 attention positions
D = 512
A = 512
V = 5000
G = 4 * D
NB = 8          # batches per core
NCORES = 8
F = 2 * D + E   # 1536
VCH = (V + 127) // 128  # 40 (last chunk has 8)

_prog_cache = {}


def _build_program():
    import concourse.bass as bass
    import concourse.bacc as bacc
    import concourse.mybir as mybir
    import concourse.tile as tile

    dt = mybir.dt
    AF = mybir.ActivationFunctionType
    ALU = mybir.AluOpType

    nc = bacc.Bacc("TRN2", target_bir_lowering=False, debug=False,
                   num_devices=NCORES)

    def din(name, shape, d=dt.bfloat16):
        return nc.dram_tensor(name, shape, d, kind="ExternalInput")

    enc_ep = din("enc_ep", [NB, E, PP])          # [b, e, p]
    enc_pe = din("enc_pe", [NB, PP, E])          # [b, p, e]
    wih1 = {0: din("wih1f", [E, G]), 1: din("wih1r", [E, G])}
    whh1 = {0: din("whh1f", [D, G]), 1: din("whh1r", [D, G])}
    wih2 = {0: din("wih2f", [2 * D, G]), 1: din("wih2r", [2 * D, G])}
    whh2 = {0: din("whh2f", [D, G]), 1: din("whh2r", [D, G])}
    b1 = {0: din("b1f", [G], dt.float32), 1: din("b1r", [G], dt.float32)}
    b2 = {0: din("b2f", [G], dt.float32), 1: din("b2r", [G], dt.float32)}
    wencT = din("wencT", [E, A])
    wdecT = din("wdecT", [2 * D, A])
    bea = din("bea", [A], dt.float32)            # benc + bdec
    wabs = din("wabs", [A], dt.float32)          # |Wfull[0]|
    sgnw = din("sgnw", [A])                      # sign(Wfull[0]) bf16
    wdiffT = din("wdiffT", [F])                  # Wg[0]-Wg[1] bf16
    bdiffs = din("bdiffs", [1, 2], dt.float32)   # [bg0-bg1, -(bg0-bg1)]
    wfcT = din("wfcT", [F, V])
    bfcp = din("bfcp", [VCH * 128], dt.float32)
    out_t = nc.dram_tensor("out", [NB, T, V], dt.float32, kind="ExternalOutput")

    with tile.TileContext(nc) as tc:
        with (
            tc.tile_pool(name="const", bufs=1) as const,
            tc.tile_pool(name="wbig", bufs=3) as wbig,
            tc.tile_pool(name="work", bufs=6) as work,
            tc.tile_pool(name="rwp", bufs=4) as rwp,
            tc.tile_pool(name="wfcp", bufs=6) as wfcp,
            tc.tile_pool(name="outp", bufs=3) as outp,
            tc.tile_pool(name="ps_g", bufs=3, space="PSUM") as ps_g,
            tc.tile_pool(name="ps_mm", bufs=3, space="PSUM") as ps_mm,
            tc.tile_pool(name="ps_sc", bufs=1, space="PSUM") as ps_sc,
        ):
            dma = nc.sync.dma_start

            # ---------------- persistent SBUF ----------------
            enc_ep_sb = const.tile([128, NB, 4, PP], dt.bfloat16)   # (b,ech,p)
            dma(out=enc_ep_sb[:],
                in_=enc_ep[:].rearrange("b (ec ep) p -> ep b ec p", ep=128))
            enc_pe_sb = const.tile([128, NB, 2, E], dt.bfloat16)    # (b,pch,e)
            dma(out=enc_pe_sb[:],
                in_=enc_pe[:].rearrange("b (pc pp) e -> pp b pc e", pp=128))

            bias1_sb, bias2_sb = {}, {}
            for d_ in (0, 1):
                bias1_sb[d_] = const.tile([128, 16], dt.float32, tag=f"b1_{d_}", name=f"b1sb{d_}")
                dma(out=bias1_sb[d_][:],
                    in_=b1[d_][:].rearrange("(c p) -> p c", p=128))
                bias2_sb[d_] = const.tile([128, 16], dt.float32, tag=f"b2_{d_}", name=f"b2sb{d_}")
                dma(out=bias2_sb[d_][:],
                    in_=b2[d_][:].rearrange("(c p) -> p c", p=128))

            wencT_sb = const.tile([128, 4, A], dt.bfloat16)   # (ech, a)
            dma(out=wencT_sb[:],
                in_=wencT[:].rearrange("(ec ep) a -> ep ec a", ep=128))
            wdecT_sb = const.tile([128, 8, A], dt.bfloat16)   # (kch, a)
            dma(out=wdecT_sb[:],
                in_=wdecT[:].rearrange("(kc kp) a -> kp kc a", kp=128))
            wabs_sb = const.tile([128, 4], dt.float32)
            dma(out=wabs_sb[:], in_=wabs[:].rearrange("(c p) -> p c", p=128))
            sgn_sb = const.tile([128, 4], dt.bfloat16)
            dma(out=sgn_sb[:], in_=sgnw[:].rearrange("(c p) -> p c", p=128))
            bea_sb = const.tile([128, 4], dt.float32)
            dma(out=bea_sb[:], in_=bea[:].rearrange("(c p) -> p c", p=128))
            wdiff_sb = const.tile([128, 12], dt.bfloat16)
            dma(out=wdiff_sb[:], in_=wdiffT[:].rearrange("(c p) -> p c", p=128))
            bdiff_sb = const.tile([1, 2], dt.float32)
            dma(out=bdiff_sb[:], in_=bdiffs[:])
            bfc_sb = const.tile([128, VCH], dt.float32)
            dma(out=bfc_sb[:], in_=bfcp[:].rearrange("(c p) -> p c", p=128))
            ones_sb = const.tile([128, 1], dt.bfloat16)
            nc.vector.memset(ones_sb[:], 1.0)

            feats = const.tile([128, 4, NB, T], dt.bfloat16)  # (ech, b, w)
            Xp1 = {d_: const.tile([128, 16, NB, T], dt.bfloat16, tag=f"xp1_{d_}", name=f"Xp1_{d_}")
                   for d_ in (0, 1)}                          # (gch, b, w)
            Xp2 = {d_: const.tile([128, 16, T, NB], dt.bfloat16, tag=f"xp2_{d_}", name=f"Xp2_{d_}")
                   for d_ in (0, 1)}                          # (gch, t, b)
            H1 = {d_: const.tile([128, 4, T, NB], dt.bfloat16, tag=f"h1_{d_}", name=f"H1_{d_}")
                  for d_ in (0, 1)}                           # (dch, t, b)
            H2 = {d_: const.tile([128, 4, T, NB], dt.bfloat16, tag=f"h2_{d_}", name=f"H2_{d_}")
                  for d_ in (0, 1)}
            att1w = const.tile([128, NB, 4, PP], dt.bfloat16)  # (b, ach, p)
            att2pb = const.tile([128, 4, 128], dt.float32)     # (ach, (b,t))
            alphaT = const.tile([128, 2, 128], dt.bfloat16)    # (pch, (b,t))
            aweT = const.tile([128, 4, 128], dt.bfloat16)      # (ech, (b,t))
            fcin = const.tile([128, 12, 128], dt.bfloat16)     # (fch, (b,t))
            E_sb = const.tile([128, 2, 128], dt.bfloat16)
            recip_sb = const.tile([1, 128], dt.float32)
            ones1_sb = const.tile([1, 128], dt.float32)
            nc.vector.memset(ones1_sb[:], 1.0)
            ones1b_sb = const.tile([1, 128], dt.bfloat16)
            nc.vector.memset(ones1b_sb[:], 1.0)

            # ---------- stage 0: feats = sum_h enc (1/16 folded in Wih1) ----
            with nc.allow_low_precision(reason="bf16 feats sum of 16 values"):
                for b_ in range(NB):
                    for ec in range(4):
                        src = enc_ep_sb[:, b_, ec, :].rearrange(
                            "p (h w) -> p w h", h=HH)
                        nc.vector.tensor_reduce(
                            out=feats[:, ec, b_, :], in_=src,
                            axis=mybir.AxisListType.X, op=ALU.add)

            # ---------- LSTM weights (stream through shared 4-slot pool) ----
            def load_w(dram, kchunks):
                # list of [128, 4, G] tiles (each 4 k-chunks) sharing one tag
                tiles = []
                for blk in range(kchunks // 4):
                    t_ = wbig.tile([128, 4, G], dt.bfloat16, tag="w",
                                   name="wtile")
                    dma(out=t_[:],
                        in_=dram[:].rearrange("(kc kp) g -> kp kc g", kp=128)
                        [:, blk * 4:(blk + 1) * 4, :])
                    tiles.append(t_)
                return tiles

            # ---------- layer-1 input projections (all t, N=128) ----------
            wih1_sb = {d_: load_w(wih1[d_], 4) for d_ in (0, 1)}
            for d_ in (0, 1):
                for mch in range(16):
                    pt = ps_mm.tile([128, 256], dt.float32, tag="pmm")
                    for kc in range(4):
                        nc.tensor.matmul(
                            pt[:, 0:128],
                            wih1_sb[d_][0][:, kc, mch * 128:(mch + 1) * 128],
                            feats[:, kc, :, :], start=(kc == 0), stop=(kc == 3))
                    nc.vector.tensor_scalar(
                        out=Xp1[d_][:, mch, :, :].rearrange("p b w -> p (b w)"),
                        in0=pt[:, 0:128], scalar1=bias1_sb[d_][:, mch:mch + 1],
                        scalar2=None, op0=ALU.add)

            whh1_sb = {d_: load_w(whh1[d_], 4) for d_ in (0, 1)}

            # ---------- LSTM cell ----------
            def cell_step(wsb, xp_slice, h_store, h_prev, c_tile, first):
                if not first:
                    pg = ps_g.tile([128, 128], dt.float32, tag="pg")
                    for mch in range(16):
                        for kc in range(4):
                            nc.tensor.matmul(
                                pg[:, mch * NB:(mch + 1) * NB],
                                wsb[:, kc, mch * 128:(mch + 1) * 128],
                                h_prev[:, kc, :],
                                start=(kc == 0), stop=(kc == 3))
                    pre = work.tile([128, 16, NB], dt.float32, tag="pre")
                    nc.vector.tensor_tensor(
                        out=pre[:],
                        in0=pg[:].rearrange("p (c b) -> p c b", b=NB),
                        in1=xp_slice, op=ALU.add)
                else:
                    pre = xp_slice
                ga = work.tile([128, 16, NB], dt.float32, tag="ga")
                nc.scalar.activation(ga[:, 0:8, :], pre[:, 0:8, :], AF.Sigmoid)
                nc.scalar.activation(ga[:, 8:12, :], pre[:, 8:12, :], AF.Tanh)
                nc.scalar.activation(ga[:, 12:16, :], pre[:, 12:16, :],
                                     AF.Sigmoid)
                ig = work.tile([128, 4, NB], dt.float32, tag="ig")
                nc.vector.tensor_tensor(out=ig[:], in0=ga[:, 0:4, :],
                                        in1=ga[:, 8:12, :], op=ALU.mult)
                if first:
                    nc.vector.tensor_copy(c_tile[:], ig[:])
                else:
                    nc.vector.tensor_tensor(out=c_tile[:], in0=c_tile[:],
                                            in1=ga[:, 4:8, :], op=ALU.mult)
                    nc.vector.tensor_tensor(out=c_tile[:], in0=c_tile[:],
                                            in1=ig[:], op=ALU.add)
                th = work.tile([128, 4, NB], dt.float32, tag="th")
                nc.scalar.activation(th[:], c_tile[:], AF.Tanh)
                nc.vector.tensor_tensor(out=h_store, in0=th[:],
                                        in1=ga[:, 12:16, :], op=ALU.mult)

            # ---------- layer-1 recurrence ----------
            c1 = {d_: work.tile([128, 4, NB], dt.float32, tag=f"c1_{d_}",
                                bufs=1, name=f"c1_{d_}") for d_ in (0, 1)}
            for s in range(T):
                for d_ in (0, 1):
                    t_log = s if d_ == 0 else T - 1 - s
                    t_prev = t_log - 1 if d_ == 0 else t_log + 1
                    cell_step(
                        whh1_sb[d_][0], Xp1[d_][:, :, :, t_log],
                        H1[d_][:, :, t_log, :],
                        None if s == 0 else H1[d_][:, :, t_prev, :],
                        c1[d_], s == 0)

            # ---------- layer-2 input projections ----------
            wih2_sb = {d_: load_w(wih2[d_], 8) for d_ in (0, 1)}
            for d_ in (0, 1):
                for mch in range(16):
                    pt = ps_mm.tile([128, 256], dt.float32, tag="pmm")
                    for kc in range(8):
                        rhs = (H1[0] if kc < 4 else H1[1])[:, kc % 4, :, :]
                        nc.tensor.matmul(
                            pt[:, 0:128],
                            wih2_sb[d_][kc // 4][:, kc % 4,
                                                 mch * 128:(mch + 1) * 128],
                            rhs, start=(kc == 0), stop=(kc == 7))
                    nc.vector.tensor_scalar(
                        out=Xp2[d_][:, mch, :, :].rearrange("p t b -> p (t b)"),
                        in0=pt[:, 0:128], scalar1=bias2_sb[d_][:, mch:mch + 1],
                        scalar2=None, op0=ALU.add)

            whh2_sb = {d_: load_w(whh2[d_], 4) for d_ in (0, 1)}

            # ---------- layer-2 recurrence ----------
            c2 = {d_: work.tile([128, 4, NB], dt.float32, tag=f"c2_{d_}",
                                bufs=1, name=f"c2_{d_}") for d_ in (0, 1)}
            for s in range(T):
                for d_ in (0, 1):
                    t_log = s if d_ == 0 else T - 1 - s
                    t_prev = t_log - 1 if d_ == 0 else t_log + 1
                    cell_step(
                        whh2_sb[d_][0], Xp2[d_][:, :, t_log, :],
                        H2[d_][:, :, t_log, :],
                        None if s == 0 else H2[d_][:, :, t_prev, :],
                        c2[d_], s == 0)

            # ---------- att2^T, +bea, scaled by |w| ----------
            def h2rhs(kc):
                return (H2[0] if kc < 4 else H2[1])[:, kc % 4, :, :] \
                    .rearrange("p t b -> p b t")

            for ac in range(4):
                pt = ps_mm.tile([128, 256], dt.float32, tag="pmm")
                for kc in range(8):
                    nc.tensor.matmul(
                        pt[:, 0:128], wdecT_sb[:, kc, ac * 128:(ac + 1) * 128],
                        h2rhs(kc), start=(kc == 0), stop=(kc == 7))
                nc.vector.tensor_scalar(
                    out=att2pb[:, ac, :], in0=pt[:, 0:128],
                    scalar1=bea_sb[:, ac:ac + 1], scalar2=wabs_sb[:, ac:ac + 1],
                    op0=ALU.add, op1=ALU.mult)

            # ---------- att1w = (Wenc^T enc) * |w| ----------
            for ac in range(4):
                for bblk in range(4):
                    pts = [ps_mm.tile([128, 256], dt.float32, tag="pmm", name="pta1")
                           for _ in range(2)]
                    for ec in range(4):
                        for bi in range(2):
                            b_ = bblk * 2 + bi
                            nc.tensor.matmul(
                                pts[bi][:],
                                wencT_sb[:, ec, ac * 128:(ac + 1) * 128],
                                enc_ep_sb[:, b_, ec, :],
                                start=(ec == 0), stop=(ec == 3))
                    for bi in range(2):
                        b_ = bblk * 2 + bi
                        nc.vector.tensor_scalar(
                            out=att1w[:, b_, ac, :], in0=pts[bi][:],
                            scalar1=wabs_sb[:, ac:ac + 1], scalar2=None,
                            op0=ALU.mult)

            # ---------- attention scores (transposed) ----------
            sc_ps = [ps_sc.tile([128, 128], dt.float32, tag=f"sc{ph}", name=f"scps{ph}")
                     for ph in range(2)]
            for b_ in range(NB):
                for tt in range(T):
                    col = b_ * T + tt
                    for ac in range(4):
                        rw = rwp.tile([128, PP], dt.bfloat16, tag="rw")
                        if col % 2 == 0:
                            nc.scalar.activation(
                                rw[:], att1w[:, b_, ac, :], AF.Relu,
                                bias=att2pb[:, ac, col:col + 1])
                        else:
                            nc.vector.tensor_scalar(
                                out=rw[:], in0=att1w[:, b_, ac, :],
                                scalar1=att2pb[:, ac, col:col + 1],
                                scalar2=0.0, op0=ALU.add, op1=ALU.max)
                        for ph in range(2):
                            nc.tensor.matmul(
                                sc_ps[ph][:, col:col + 1],
                                rw[:, ph * 128:(ph + 1) * 128],
                                sgn_sb[:, ac:ac + 1],
                                start=(ac == 0), stop=(ac == 3))

            # ---------- softmax over p (stay transposed) ----------
            for ph in range(2):
                nc.scalar.activation(E_sb[:, ph, :], sc_ps[ph][:], AF.Exp)
            sums = ps_sc.tile([1, 128], dt.float32, tag="sc0")
            for ph in range(2):
                nc.tensor.matmul(sums[:], ones_sb[:], E_sb[:, ph, :],
                                 start=(ph == 0), stop=(ph == 1))
            nc.vector.reciprocal(recip_sb[:], sums[:])
            recip_bc = ps_g.tile([128, 128], dt.float32, tag="pg",
                                 name="recip_bc")
            nc.tensor.matmul(recip_bc[:], ones1_sb[:], recip_sb[:],
                             start=True, stop=True)
            for ph in range(2):
                nc.vector.tensor_tensor(out=alphaT[:, ph, :],
                                        in0=E_sb[:, ph, :],
                                        in1=recip_bc[:], op=ALU.mult)

            # ---------- awe^T[e,(b,t)] ----------
            for ec in range(4):
                pa = ps_g.tile([128, 128], dt.float32, tag="pg")
                for b_ in range(NB):
                    for pc in range(2):
                        nc.tensor.matmul(
                            pa[:, b_ * T:(b_ + 1) * T],
                            enc_pe_sb[:, b_, pc, ec * 128:(ec + 1) * 128],
                            alphaT[:, pc, b_ * T:(b_ + 1) * T],
                            start=(pc == 0), stop=(pc == 1))
                nc.vector.tensor_copy(aweT[:, ec, :], pa[:])

            # ---------- gate ----------
            def fc_feat_rhs(kc):
                return h2rhs(kc) if kc < 8 else aweT[:, kc - 8, :]

            gl = ps_sc.tile([1, 128], dt.float32, tag="sc1")
            for kc in range(12):
                nc.tensor.matmul(gl[:], wdiff_sb[:, kc:kc + 1], fc_feat_rhs(kc),
                                 start=(kc == 0), stop=(kc == 11))
            g0 = work.tile([1, 128], dt.bfloat16, tag="g0", bufs=1)
            g1 = work.tile([1, 128], dt.bfloat16, tag="g1", bufs=1)
            nc.scalar.activation(g0[:], gl[:], AF.Sigmoid, bias=bdiff_sb[:, 0:1])
            nc.scalar.activation(g1[:], gl[:], AF.Sigmoid, bias=bdiff_sb[:, 1:2],
                                 scale=-1.0)
            g0b = ps_g.tile([128, 128], dt.float32, tag="pg", name="g0b")
            g1b = ps_g.tile([128, 128], dt.float32, tag="pg", name="g1b")
            nc.tensor.matmul(g0b[:], ones1b_sb[:], g0[:], start=True, stop=True)
            nc.tensor.matmul(g1b[:], ones1b_sb[:], g1[:], start=True, stop=True)

            # ---------- fc_in = [g0*hidden ; g1*awe] ----------
            for kc in range(12):
                nc.vector.tensor_tensor(
                    out=fcin[:, kc, :], in0=fc_feat_rhs(kc),
                    in1=(g0b if kc < 8 else g1b)[:], op=ALU.mult)

            # ---------- fc ----------
            for vc in range(VCH):
                vn = min(128, V - vc * 128)
                wt = wfcp.tile([128, 12, 128], dt.bfloat16, tag="wfc")
                dma(out=wt[:, :, 0:vn],
                    in_=wfcT[:, vc * 128:vc * 128 + vn]
                    .rearrange("(kc kp) v -> kp kc v", kp=128))
                pt = ps_mm.tile([128, 256], dt.float32, tag="pmm")
                for kc in range(12):
                    nc.tensor.matmul(pt[0:vn, 0:128], wt[:, kc, 0:vn],
                                     fcin[:, kc, :], start=(kc == 0),
                                     stop=(kc == 11))
                ost = outp.tile([128, 128], dt.float32, tag="ost")
                nc.vector.tensor_scalar(
                    out=ost[0:vn, :], in0=pt[0:vn, 0:128],
                    scalar1=bfc_sb[0:vn, vc:vc + 1], scalar2=None, op0=ALU.add)
                dst = bass.AP(tensor=out_t[:].tensor, offset=vc * 128,
                              ap=[[1, vn], [T * V, NB], [V, T]])
                dma(out=dst,
                    in_=ost[0:vn, :].rearrange("v (b t) -> v b t", b=NB))

    nc.compile()
    return nc


def _host_prep(inputs):
    f32 = np.float32

    def bf(x):
        return np.ascontiguousarray(np.asarray(x, f32).astype(BF))

    enc = np.asarray(inputs["encoder_out"], f32)
    enc_p = enc.reshape(B, E, PP)

    common = {}
    common["wih1f"] = bf(np.asarray(inputs["Wih1"], f32).T / HH)
    common["wih1r"] = bf(np.asarray(inputs["Wih1r"], f32).T / HH)
    common["whh1f"] = bf(np.asarray(inputs["Whh1"], f32).T)
    common["whh1r"] = bf(np.asarray(inputs["Whh1r"], f32).T)
    common["wih2f"] = bf(np.asarray(inputs["Wih2"], f32).T)
    common["wih2r"] = bf(np.asarray(inputs["Wih2r"], f32).T)
    common["whh2f"] = bf(np.asarray(inputs["Whh2"], f32).T)
    common["whh2r"] = bf(np.asarray(inputs["Whh2r"], f32).T)
    common["b1f"] = np.asarray(inputs["bih1"] + inputs["bhh1"], f32)
    common["b1r"] = np.asarray(inputs["bih1r"] + inputs["bhh1r"], f32)
    common["b2f"] = np.asarray(inputs["bih2"] + inputs["bhh2"], f32)
    common["b2r"] = np.asarray(inputs["bih2r"] + inputs["bhh2r"], f32)
    common["wencT"] = bf(np.asarray(inputs["Wenc"], f32).T)
    common["wdecT"] = bf(np.asarray(inputs["Wdec"], f32).T)
    common["bea"] = np.asarray(inputs["benc"] + inputs["bdec"], f32)
    wf = np.asarray(inputs["Wfull"], f32)[0]
    common["wabs"] = np.abs(wf).astype(f32)
    common["sgnw"] = bf(np.where(wf >= 0, 1.0, -1.0))
    wg = np.asarray(inputs["Wg"], f32)
    common["wdiffT"] = bf(wg[0] - wg[1])
    bd = float(np.asarray(inputs["bg"], f32)[0] - np.asarray(inputs["bg"], f32)[1])
    common["bdiffs"] = np.array([[bd, -bd]], f32)
    common["wfcT"] = bf(np.asarray(inputs["Wfc"], f32).T)
    bfc = np.zeros(VCH * 128, f32)
    bfc[:V] = np.asarray(inputs["bfc"], f32)
    common["bfcp"] = bfc

    in_maps = []
    for c in range(NCORES):
        m = dict(common)
        sl = enc_p[c * NB:(c + 1) * NB]
        m["enc_ep"] = bf(sl)
        m["enc_pe"] = bf(np.ascontiguousarray(sl.transpose(0, 2, 1)))
        in_maps.append(m)
    return in_maps


def _get_program():
    if "nc" not in _prog_cache:
        _prog_cache["nc"] = _build_program()
    return _prog_cache["nc"]


def kernel(**inputs):
    from concourse.bass_utils import run_bass_kernel_spmd

    nc = _get_program()
    in_maps = _host_prep(inputs)
    res = run_bass_kernel_spmd(nc, in_maps, list(range(NCORES)))
    # per-core result is [b, t, v]; assemble to (T, B, V)
    out = np.concatenate(
        [res.results[c]["out"].transpose(1, 0, 2) for c in range(NCORES)],
        axis=1)
    return np.ascontiguousarray(out, np.float32)


# revision 14
# speedup vs baseline: 206.9680x; 206.9680x over previous
"""Trainium2 Bass kernel for DecoderWithAttention (bidirectional 2-layer LSTM +
additive attention + gated fc), data-parallel over batch across 8 NeuronCores.

Shapes (hardcoded): encoder_out (64, 512, 16, 16), T=16, D=A=512, V=5000.
Per core: 8 batches, full network, weights replicated (no collectives available
under this axon terminal, so each core is fully independent).

Key layout decisions (per core):
  - All matmuls weight-stationary: matmul(out, lhsT, rhs): out = lhsT^T @ rhs.
  - LSTM gates PSUM: [128 part = gate%128, cols = (gate_chunk 16, batch 8)].
  - Input projections for all 16 steps batched (N=128); only Whh per step.
  - Hidden stores H*: [128, dch(4), t(16), b(8)] bf16, logical-t order (the
    reverse cells index t=15-s at compile time, so no data reversal anywhere).
  - Attention in transposed layout (A on partitions). relu(x)@Wfull uses
    relu(x)*w = sgn(w)*relu(x*|w|): |w| folded into ACT scale / precomputed
    tiles, sgn(w) as the PE reduction rhs. Softmax over p via PE ones-sum in
    [p, (b,t)] layout, no max subtraction (|score| bounded), bfull dropped
    (softmax shift invariance).
  - gate softmax(2) == sigmoid(logit diff), Wg[0]-Wg[1] folded host-side.
  - Mean over H folded into Wih1 (1/16); bih+bhh folded host-side.
"""

import numpy as np
import ml_dtypes

BF = ml_dtypes.bfloat16
B, E, HH, WW = 64, 512, 16, 16
T = WW          # 16 timesteps
PP = HH * WW    # 256 attention positions
D = 512
A = 512
V = 5000
G = 4 * D
NB = 8          # batches per core
NCORES = 8
F = 2 * D + E   # 1536
VCH = (V + 127) // 128  # 40 (last chunk has 8)

_prog_cache = {}


def _build_program():
    import concourse.bass as bass
    import concourse.bacc as bacc
    import concourse.mybir as mybir
    import concourse.tile as tile

    dt = mybir.dt
    AF = mybir.ActivationFunctionType
    ALU = mybir.AluOpType

    nc = bacc.Bacc("TRN2", target_bir_lowering=False, debug=False,
                   num_devices=NCORES)

    def din(name, shape, d=dt.bfloat16):
        return nc.dram_tensor(name, shape, d, kind="ExternalInput")

    enc_ep = din("enc_ep", [NB, E, PP])          # [b, e, p]
    enc_pe = din("enc_pe", [NB, PP, E])          # [b, p, e]
    wih1 = {0: din("wih1f", [E, G]), 1: din("wih1r", [E, G])}
    whh1 = {0: din("whh1f", [D, G]), 1: din("whh1r", [D, G])}
    wih2 = {0: din("wih2f", [2 * D, G]), 1: din("wih2r", [2 * D, G])}
    whh2 = {0: din("whh2f", [D, G]), 1: din("whh2r", [D, G])}
    b1 = {0: din("b1f", [G], dt.float32), 1: din("b1r", [G], dt.float32)}
    b2 = {0: din("b2f", [G], dt.float32), 1: din("b2r", [G], dt.float32)}
    wencT = din("wencT", [E, A])
    wdecT = din("wdecT", [2 * D, A])
    bea = din("bea", [A], dt.float32)            # benc + bdec
    wabs = din("wabs", [A], dt.float32)          # |Wfull[0]|
    sgnw = din("sgnw", [A])                      # sign(Wfull[0]) bf16
    wdiffT = din("wdiffT", [F])                  # Wg[0]-Wg[1] bf16
    bdiffs = din("bdiffs", [1, 2], dt.float32)   # [bg0-bg1, -(bg0-bg1)]
    wfcT = din("wfcT", [F, V])
    bfcp = din("bfcp", [VCH * 128], dt.float32)
    out_t = nc.dram_tensor("out", [NB, T, V], dt.float32, kind="ExternalOutput")

    with tile.TileContext(nc) as tc:
        with (
            tc.tile_pool(name="const", bufs=1) as const,
            tc.tile_pool(name="wbig", bufs=3) as wbig,
            tc.tile_pool(name="work", bufs=12) as work,
            tc.tile_pool(name="rwp", bufs=12) as rwp,
            tc.tile_pool(name="wfcp", bufs=8) as wfcp,
            tc.tile_pool(name="outp", bufs=6) as outp,
            tc.tile_pool(name="ps_g", bufs=3, space="PSUM") as ps_g,
            tc.tile_pool(name="ps_mm", bufs=3, space="PSUM") as ps_mm,
            tc.tile_pool(name="ps_sc", bufs=1, space="PSUM") as ps_sc,
        ):
            dma = nc.sync.dma_start

            # ---------------- persistent SBUF ----------------
            enc_ep_sb = const.tile([128, NB, 4, PP], dt.bfloat16)   # (b,ech,p)
            dma(out=enc_ep_sb[:],
                in_=enc_ep[:].rearrange("b (ec ep) p -> ep b ec p", ep=128))
            enc_pe_sb = const.tile([128, NB, 2, E], dt.bfloat16)    # (b,pch,e)
            dma(out=enc_pe_sb[:],
                in_=enc_pe[:].rearrange("b (pc pp) e -> pp b pc e", pp=128))

            bias1_sb, bias2_sb = {}, {}
            for d_ in (0, 1):
                bias1_sb[d_] = const.tile([128, 16], dt.float32, tag=f"b1_{d_}", name=f"b1sb{d_}")
                dma(out=bias1_sb[d_][:],
                    in_=b1[d_][:].rearrange("(c p) -> p c", p=128))
                bias2_sb[d_] = const.tile([128, 16], dt.float32, tag=f"b2_{d_}", name=f"b2sb{d_}")
                dma(out=bias2_sb[d_][:],
                    in_=b2[d_][:].rearrange("(c p) -> p c", p=128))

            wencT_sb = const.tile([128, 4, A], dt.bfloat16)   # (ech, a)
            dma(out=wencT_sb[:],
                in_=wencT[:].rearrange("(ec ep) a -> ep ec a", ep=128))
            wdecT_sb = const.tile([128, 8, A], dt.bfloat16)   # (kch, a)
            dma(out=wdecT_sb[:],
                in_=wdecT[:].rearrange("(kc kp) a -> kp kc a", kp=128))
            wabs_sb = const.tile([128, 4], dt.float32)
            dma(out=wabs_sb[:], in_=wabs[:].rearrange("(c p) -> p c", p=128))
            sgn_sb = const.tile([128, 4], dt.bfloat16)
            dma(out=sgn_sb[:], in_=sgnw[:].rearrange("(c p) -> p c", p=128))
            bea_sb = const.tile([128, 4], dt.float32)
            dma(out=bea_sb[:], in_=bea[:].rearrange("(c p) -> p c", p=128))
            wdiff_sb = const.tile([128, 12], dt.bfloat16)
            dma(out=wdiff_sb[:], in_=wdiffT[:].rearrange("(c p) -> p c", p=128))
            bdiff_sb = const.tile([1, 2], dt.float32)
            dma(out=bdiff_sb[:], in_=bdiffs[:])
            bfc_sb = const.tile([128, VCH], dt.float32)
            dma(out=bfc_sb[:], in_=bfcp[:].rearrange("(c p) -> p c", p=128))
            ones_sb = const.tile([128, 1], dt.bfloat16)
            nc.vector.memset(ones_sb[:], 1.0)

            feats = const.tile([128, 4, NB, T], dt.bfloat16)  # (ech, b, w)
            Xp1 = {d_: const.tile([128, 16, NB, T], dt.bfloat16, tag=f"xp1_{d_}", name=f"Xp1_{d_}")
                   for d_ in (0, 1)}                          # (gch, b, w)
            Xp2 = {d_: const.tile([128, 16, T, NB], dt.bfloat16, tag=f"xp2_{d_}", name=f"Xp2_{d_}")
                   for d_ in (0, 1)}                          # (gch, t, b)
            H1 = {d_: const.tile([128, 4, T, NB], dt.bfloat16, tag=f"h1_{d_}", name=f"H1_{d_}")
                  for d_ in (0, 1)}                           # (dch, t, b)
            H2 = {d_: const.tile([128, 4, T, NB], dt.bfloat16, tag=f"h2_{d_}", name=f"H2_{d_}")
                  for d_ in (0, 1)}
            att1w = const.tile([128, NB, 4, PP], dt.bfloat16)  # (b, ach, p)
            att2pb = const.tile([128, 4, 128], dt.float32)     # (ach, (b,t))
            alphaT = const.tile([128, 2, 128], dt.bfloat16)    # (pch, (b,t))
            aweT = const.tile([128, 4, 128], dt.bfloat16)      # (ech, (b,t))
            fcin = const.tile([128, 12, 128], dt.bfloat16)     # (fch, (b,t))
            E_sb = const.tile([128, 2, 128], dt.bfloat16)
            recip_sb = const.tile([1, 128], dt.float32)
            ones1_sb = const.tile([1, 128], dt.float32)
            nc.vector.memset(ones1_sb[:], 1.0)
            ones1b_sb = const.tile([1, 128], dt.bfloat16)
            nc.vector.memset(ones1b_sb[:], 1.0)

            # ---------- stage 0: feats = sum_h enc (1/16 folded in Wih1) ----
            with nc.allow_low_precision(reason="bf16 feats sum of 16 values"):
                for b_ in range(NB):
                    rsrc = enc_ep_sb[:, b_, :, :].rearrange(
                        "p ec (h w) -> p ec w h", h=HH)
                    nc.vector.tensor_reduce(
                        out=feats[:, :, b_, :], in_=rsrc,
                        axis=mybir.AxisListType.X, op=ALU.add)

            # ---------- LSTM weights (stream through shared 4-slot pool) ----
            def load_w(dram, kchunks):
                # list of [128, 4, G] tiles (each 4 k-chunks) sharing one tag
                tiles = []
                for blk in range(kchunks // 4):
                    t_ = wbig.tile([128, 4, G], dt.bfloat16, tag="w",
                                   name="wtile")
                    dma(out=t_[:],
                        in_=dram[:].rearrange("(kc kp) g -> kp kc g", kp=128)
                        [:, blk * 4:(blk + 1) * 4, :])
                    tiles.append(t_)
                return tiles

            # ---------- layer-1 input projections (all t, N=128) ----------
            wih1_sb = {d_: load_w(wih1[d_], 4) for d_ in (0, 1)}
            for d_ in (0, 1):
                for mch in range(16):
                    pt = ps_mm.tile([128, 512], dt.float32, tag="pmm")
                    for kc in range(4):
                        nc.tensor.matmul(
                            pt[:, 0:128],
                            wih1_sb[d_][0][:, kc, mch * 128:(mch + 1) * 128],
                            feats[:, kc, :, :], start=(kc == 0), stop=(kc == 3))
                    nc.vector.tensor_scalar(
                        out=Xp1[d_][:, mch, :, :].rearrange("p b w -> p (b w)"),
                        in0=pt[:, 0:128], scalar1=bias1_sb[d_][:, mch:mch + 1],
                        scalar2=None, op0=ALU.add)

            whh1_sb = {d_: load_w(whh1[d_], 4) for d_ in (0, 1)}
            whh1_view = {d_: whh1_sb[d_][0] for d_ in (0, 1)}

            # ---------- LSTM fused step pair ----------
            # Gate blocks host-permuted to (i, f, o, g):
            # ch 0-3=i, 4-7=f, 8-11=o, 12-15=g.
            # psum/pre/ga layout: [128, cell(2), ch(16), b(8)]; both cells'
            # elementwise fused into single ops (DVE/ACT ops are the scarce
            # resource on this platform).
            def step_pair(wsb, xps, Hs, c_tile, s, lgi):
                pg = ps_g.tile([128, 2, 16, NB], dt.float32, tag="pg",
                               name="pg")
                pre = work.tile([128, 2, 16, NB], dt.float32, tag="pre",
                                name="pre")
                for d_ in (0, 1):
                    t_log = s if d_ == 0 else T - 1 - s
                    t_prev = t_log - 1 if d_ == 0 else t_log + 1
                    if s > 0:
                        h_prev = Hs[d_][:, :, t_prev, :]
                        for mch in range(16):
                            for kc in range(4):
                                nc.tensor.matmul(
                                    pg[:, d_, mch, :],
                                    wsb[d_][:, kc, mch * 128:(mch + 1) * 128],
                                    h_prev[:, kc, :],
                                    start=(kc == 0), stop=(kc == 3))
                        nc.vector.tensor_tensor(
                            out=pre[:, d_, :, :], in0=pg[:, d_, :, :],
                            in1=xps[d_], op=ALU.add)
                    else:
                        nc.vector.tensor_copy(pre[:, d_, :, :], xps[d_])
                ga = work.tile([128, 2, 16, NB], dt.float32, tag="ga",
                               name="ga")
                nc.scalar.activation(ga[:, :, 0:12, :], pre[:, :, 0:12, :],
                                     AF.Sigmoid)
                nc.scalar.activation(ga[:, :, 12:16, :], pre[:, :, 12:16, :],
                                     AF.Tanh)
                ig = work.tile([128, 2, 4, NB], dt.float32, tag="ig",
                               name="ig")
                nc.vector.tensor_tensor(out=ig[:], in0=ga[:, :, 0:4, :],
                                        in1=ga[:, :, 12:16, :], op=ALU.mult)
                if s == 0:
                    nc.vector.tensor_copy(c_tile[:], ig[:])
                else:
                    nc.vector.tensor_tensor(out=c_tile[:], in0=c_tile[:],
                                            in1=ga[:, :, 4:8, :], op=ALU.mult)
                    nc.vector.tensor_tensor(out=c_tile[:], in0=c_tile[:],
                                            in1=ig[:], op=ALU.add)
                th = work.tile([128, 2, 4, NB], dt.float32, tag="th",
                               name="th")
                nc.scalar.activation(th[:], c_tile[:], AF.Tanh)
                for d_ in (0, 1):
                    t_log = s if d_ == 0 else T - 1 - s
                    nc.vector.tensor_tensor(out=Hs[d_][:, :, t_log, :],
                                            in0=th[:, d_, :, :],
                                            in1=ga[:, d_, 8:12, :],
                                            op=ALU.mult)

            # ---------- layer-1 recurrence ----------
            c1 = work.tile([128, 2, 4, NB], dt.float32, tag="c1", bufs=1,
                           name="c1")
            for s in range(T):
                step_pair(whh1_view, {
                    0: Xp1[0][:, :, :, s],
                    1: Xp1[1][:, :, :, T - 1 - s]}, H1, c1, s, 1)

            # ---------- layer-2 input projections ----------
            wih2_sb = {d_: load_w(wih2[d_], 8) for d_ in (0, 1)}
            for d_ in (0, 1):
                for mch in range(16):
                    pt = ps_mm.tile([128, 512], dt.float32, tag="pmm")
                    for kc in range(8):
                        rhs = (H1[0] if kc < 4 else H1[1])[:, kc % 4, :, :]
                        nc.tensor.matmul(
                            pt[:, 0:128],
                            wih2_sb[d_][kc // 4][:, kc % 4,
                                                 mch * 128:(mch + 1) * 128],
                            rhs, start=(kc == 0), stop=(kc == 7))
                    nc.vector.tensor_scalar(
                        out=Xp2[d_][:, mch, :, :].rearrange("p t b -> p (t b)"),
                        in0=pt[:, 0:128], scalar1=bias2_sb[d_][:, mch:mch + 1],
                        scalar2=None, op0=ALU.add)

            whh2_sb = {d_: load_w(whh2[d_], 4) for d_ in (0, 1)}
            whh2_view = {d_: whh2_sb[d_][0] for d_ in (0, 1)}

            # ---------- layer-2 recurrence ----------
            c2 = work.tile([128, 2, 4, NB], dt.float32, tag="c2", bufs=1,
                           name="c2")
            for s in range(T):
                step_pair(whh2_view, {
                    0: Xp2[0][:, :, s, :],
                    1: Xp2[1][:, :, T - 1 - s, :]}, H2, c2, s, 2)

            # ---------- att2^T, +bea, scaled by |w| ----------
            def h2rhs(kc):
                return (H2[0] if kc < 4 else H2[1])[:, kc % 4, :, :] \
                    .rearrange("p t b -> p b t")

            for ac in range(4):
                pt = ps_mm.tile([128, 512], dt.float32, tag="pmm")
                for kc in range(8):
                    nc.tensor.matmul(
                        pt[:, 0:128], wdecT_sb[:, kc, ac * 128:(ac + 1) * 128],
                        h2rhs(kc), start=(kc == 0), stop=(kc == 7))
                nc.vector.tensor_scalar(
                    out=att2pb[:, ac, :], in0=pt[:, 0:128],
                    scalar1=bea_sb[:, ac:ac + 1], scalar2=wabs_sb[:, ac:ac + 1],
                    op0=ALU.add, op1=ALU.mult)

            # ---------- att1w = (Wenc*|w|)^T enc  (|w| folded host-side) --
            for ac in range(4):
                for bblk in range(4):
                    pt = ps_mm.tile([128, 512], dt.float32, tag="pmm",
                                    name="pta1")
                    for ec in range(4):
                        nc.tensor.matmul(
                            pt[:],
                            wencT_sb[:, ec, ac * 128:(ac + 1) * 128],
                            enc_ep_sb[:, 2 * bblk:2 * bblk + 2, ec, :],
                            start=(ec == 0), stop=(ec == 3))
                    nc.vector.tensor_copy(
                        att1w[:, 2 * bblk:2 * bblk + 2, ac, :], pt[:])

            # ---------- attention scores (transposed) ----------
            sc_ps = [ps_sc.tile([128, 128], dt.float32, tag=f"sc{ph}", name=f"scps{ph}")
                     for ph in range(2)]
            for b_ in range(NB):
                for tt in range(T):
                    col = b_ * T + tt
                    for ac in range(4):
                        rw = rwp.tile([128, PP], dt.bfloat16, tag="rw")
                        if col % 2 == 0:
                            nc.scalar.activation(
                                rw[:], att1w[:, b_, ac, :], AF.Relu,
                                bias=att2pb[:, ac, col:col + 1])
                        else:
                            nc.vector.tensor_scalar(
                                out=rw[:], in0=att1w[:, b_, ac, :],
                                scalar1=att2pb[:, ac, col:col + 1],
                                scalar2=0.0, op0=ALU.add, op1=ALU.max)
                        for ph in range(2):
                            nc.tensor.matmul(
                                sc_ps[ph][:, col:col + 1],
                                rw[:, ph * 128:(ph + 1) * 128],
                                sgn_sb[:, ac:ac + 1],
                                start=(ac == 0), stop=(ac == 3))

            # ---------- softmax over p (stay transposed) ----------
            for ph in range(2):
                nc.scalar.activation(E_sb[:, ph, :], sc_ps[ph][:], AF.Exp)
            sums = ps_sc.tile([1, 128], dt.float32, tag="sc0")
            for ph in range(2):
                nc.tensor.matmul(sums[:], ones_sb[:], E_sb[:, ph, :],
                                 start=(ph == 0), stop=(ph == 1))
            nc.vector.reciprocal(recip_sb[:], sums[:])
            recip_bc = ps_g.tile([128, 128], dt.float32, tag="pg",
                                 name="recip_bc")
            nc.tensor.matmul(recip_bc[:], ones1_sb[:], recip_sb[:],
                             start=True, stop=True)
            for ph in range(2):
                nc.vector.tensor_tensor(out=alphaT[:, ph, :],
                                        in0=E_sb[:, ph, :],
                                        in1=recip_bc[:], op=ALU.mult)

            # ---------- awe^T[e,(b,t)] ----------
            for ec in range(4):
                pa = ps_g.tile([128, 128], dt.float32, tag="pg")
                for b_ in range(NB):
                    for pc in range(2):
                        nc.tensor.matmul(
                            pa[:, b_ * T:(b_ + 1) * T],
                            enc_pe_sb[:, b_, pc, ec * 128:(ec + 1) * 128],
                            alphaT[:, pc, b_ * T:(b_ + 1) * T],
                            start=(pc == 0), stop=(pc == 1))
                nc.vector.tensor_copy(aweT[:, ec, :], pa[:])

            # ---------- gate ----------
            def fc_feat_rhs(kc):
                return h2rhs(kc) if kc < 8 else aweT[:, kc - 8, :]

            gl = ps_sc.tile([1, 128], dt.float32, tag="sc1")
            for kc in range(12):
                nc.tensor.matmul(gl[:], wdiff_sb[:, kc:kc + 1], fc_feat_rhs(kc),
                                 start=(kc == 0), stop=(kc == 11))
            g0 = work.tile([1, 128], dt.bfloat16, tag="g0", bufs=1)
            g1 = work.tile([1, 128], dt.bfloat16, tag="g1", bufs=1)
            nc.scalar.activation(g0[:], gl[:], AF.Sigmoid, bias=bdiff_sb[:, 0:1])
            nc.scalar.activation(g1[:], gl[:], AF.Sigmoid, bias=bdiff_sb[:, 1:2],
                                 scale=-1.0)
            g0b = ps_g.tile([128, 128], dt.float32, tag="pg", name="g0b")
            g1b = ps_g.tile([128, 128], dt.float32, tag="pg", name="g1b")
            nc.tensor.matmul(g0b[:], ones1b_sb[:], g0[:], start=True, stop=True)
            nc.tensor.matmul(g1b[:], ones1b_sb[:], g1[:], start=True, stop=True)

            # ---------- fc_in = [g0*hidden ; g1*awe] ----------
            for kc in range(12):
                nc.vector.tensor_tensor(
                    out=fcin[:, kc, :], in0=fc_feat_rhs(kc),
                    in1=(g0b if kc < 8 else g1b)[:], op=ALU.mult)

            # ---------- fc ----------
            for vc in range(VCH):
                vn = min(128, V - vc * 128)
                wt = wfcp.tile([128, 12, 128], dt.bfloat16, tag="wfc")
                dma(out=wt[:, :, 0:vn],
                    in_=wfcT[:, vc * 128:vc * 128 + vn]
                    .rearrange("(kc kp) v -> kp kc v", kp=128))
                pt = ps_mm.tile([128, 512], dt.float32, tag="pmm")
                for kc in range(12):
                    nc.tensor.matmul(pt[0:vn, 0:128], wt[:, kc, 0:vn],
                                     fcin[:, kc, :], start=(kc == 0),
                                     stop=(kc == 11))
                ost = outp.tile([128, 128], dt.float32, tag="ost")
                nc.vector.tensor_scalar(
                    out=ost[0:vn, :], in0=pt[0:vn, 0:128],
                    scalar1=bfc_sb[0:vn, vc:vc + 1], scalar2=None, op0=ALU.add)
                dst = bass.AP(tensor=out_t[:].tensor, offset=vc * 128,
                              ap=[[1, vn], [T * V, NB], [V, T]])
                dma(out=dst,
                    in_=ost[0:vn, :].rearrange("v (b t) -> v b t", b=NB))

    nc.compile()
    return nc


def _host_prep(inputs):
    f32 = np.float32

    def bf(x):
        return np.ascontiguousarray(np.asarray(x, f32).astype(BF))

    enc = np.asarray(inputs["encoder_out"], f32)
    enc_p = enc.reshape(B, E, PP)

    # permute gate blocks (i,f,g,o) -> (i,f,o,g) so one sigmoid spans i,f,o
    gp = np.r_[0:2 * D, 3 * D:4 * D, 2 * D:3 * D]

    common = {}
    common["wih1f"] = bf(np.asarray(inputs["Wih1"], f32).T[:, gp] / HH)
    common["wih1r"] = bf(np.asarray(inputs["Wih1r"], f32).T[:, gp] / HH)
    common["whh1f"] = bf(np.asarray(inputs["Whh1"], f32).T[:, gp])
    common["whh1r"] = bf(np.asarray(inputs["Whh1r"], f32).T[:, gp])
    common["wih2f"] = bf(np.asarray(inputs["Wih2"], f32).T[:, gp])
    common["wih2r"] = bf(np.asarray(inputs["Wih2r"], f32).T[:, gp])
    common["whh2f"] = bf(np.asarray(inputs["Whh2"], f32).T[:, gp])
    common["whh2r"] = bf(np.asarray(inputs["Whh2r"], f32).T[:, gp])
    common["b1f"] = np.asarray(inputs["bih1"] + inputs["bhh1"], f32)[gp]
    common["b1r"] = np.asarray(inputs["bih1r"] + inputs["bhh1r"], f32)[gp]
    common["b2f"] = np.asarray(inputs["bih2"] + inputs["bhh2"], f32)[gp]
    common["b2r"] = np.asarray(inputs["bih2r"] + inputs["bhh2r"], f32)[gp]
    common["wencT"] = bf(np.asarray(inputs["Wenc"], f32).T
                         * np.abs(np.asarray(inputs["Wfull"], f32)[0])[None, :])
    common["wdecT"] = bf(np.asarray(inputs["Wdec"], f32).T)
    common["bea"] = np.asarray(inputs["benc"] + inputs["bdec"], f32)
    wf = np.asarray(inputs["Wfull"], f32)[0]
    common["wabs"] = np.abs(wf).astype(f32)
    common["sgnw"] = bf(np.where(wf >= 0, 1.0, -1.0))
    wg = np.asarray(inputs["Wg"], f32)
    common["wdiffT"] = bf(wg[0] - wg[1])
    bd = float(np.asarray(inputs["bg"], f32)[0] - np.asarray(inputs["bg"], f32)[1])
    common["bdiffs"] = np.array([[bd, -bd]], f32)
    common["wfcT"] = bf(np.asarray(inputs["Wfc"], f32).T)
    bfc = np.zeros(VCH * 128, f32)
    bfc[:V] = np.asarray(inputs["bfc"], f32)
    common["bfcp"] = bfc

    in_maps = []
    for c in range(NCORES):
        m = dict(common)
        sl = enc_p[c * NB:(c + 1) * NB]
        m["enc_ep"] = bf(sl)
        m["enc_pe"] = bf(np.ascontiguousarray(sl.transpose(0, 2, 1)))
        in_maps.append(m)
    return in_maps


def _get_program():
    if "nc" not in _prog_cache:
        _prog_cache["nc"] = _build_program()
    return _prog_cache["nc"]


def kernel(**inputs):
    from concourse.bass_utils import run_bass_kernel_spmd

    nc = _get_program()
    in_maps = _host_prep(inputs)
    res = run_bass_kernel_spmd(nc, in_maps, list(range(NCORES)))
    # per-core result is [b, t, v]; assemble to (T, B, V)
    out = np.concatenate(
        [res.results[c]["out"].transpose(1, 0, 2) for c in range(NCORES)],
        axis=1)
    return np.ascontiguousarray(out, np.float32)
